# revision 9
# baseline (speedup 1.0000x reference)
"""Chunkwise causal attention (full causal MHA + QKV/out projections) on 8 trn2 cores.

Sharding: data-parallel over batch (B=2) x tensor-parallel over heads (16 -> 4 per
core). Each core computes, for its (batch, 4-head group):
  qkv projection (f32r matmuls), causal attention (f32r QK, fp16 PV with a fused
  ones-column providing softmax denominators), and its partial out-projection
  (fp16). Host sums the 4 partials per batch and adds biases where needed.

Self-contained: hardcodes all shapes from the problem spec.
"""

import numpy as np

import concourse.bass as bass
import concourse.mybir as mybir
import concourse.tile as tile
from concourse import bacc
from concourse.bass_utils import run_bass_kernel_spmd
from concourse.masks import make_identity

# Problem shapes
B, S, D = 2, 2048, 2048
H, Dh = 16, 128
HC = 4                      # heads per core
P = 128
SPLIT = 4
SQ = S // SPLIT             # 512: S-chunk processed per outer phase
N_DC = D // P               # 16 contraction chunks for projections
N_SC = S // P               # 16 S chunks
SCALE = 1.0 / float(np.sqrt(Dh))
VW = 132                    # v_aug row width (129 used: 128 dh + 1 ones col)

f32 = mybir.dt.float32
f32r = mybir.dt.float32r
f16 = mybir.dt.float16

_COMPILED = {}

# Tunable pool sizes (PSUM pools must keep total banks <= 8)
CFG = dict(WBUFS=3, EXPBUFS=18, QKPSB=2, TRB=2, QK2B=2, PVB=2, SPB=4, OSB=3)


def build_program():
    nc = bacc.Bacc("TRN2", target_bir_lowering=False, debug=False)

    xT = nc.dram_tensor("xT", (D, S), f32r, kind="ExternalInput")
    wq = nc.dram_tensor("wq", (D, HC * Dh), f32r, kind="ExternalInput")
    wk = nc.dram_tensor("wk", (D, HC * Dh), f32r, kind="ExternalInput")
    wv = nc.dram_tensor("wv", (D, HC * Dh), f32r, kind="ExternalInput")
    wout = nc.dram_tensor("wout", (HC * Dh, D), f16, kind="ExternalInput")
    bq = nc.dram_tensor("bq", (P, HC), f32, kind="ExternalInput")
    bk = nc.dram_tensor("bk", (P, HC), f32, kind="ExternalInput")
    bvb = nc.dram_tensor("bvb", (P, HC * Dh), f32, kind="ExternalInput")
    masks = nc.dram_tensor("masks", (4, P, 512), f16, kind="ExternalInput")
    outp = nc.dram_tensor("outp", (S, D), f32, kind="ExternalOutput")

    xT_ap, wq_ap, wk_ap, wv_ap = xT.ap(), wq.ap(), wk.ap(), wv.ap()
    wout_ap, masks_ap, outp_ap = wout.ap(), masks.ap(), outp.ap()

    with tile.TileContext(nc, trace_sim=CFG.get('TRACE', False)) as tc:
        with tc.tile_pool(name="const", bufs=1) as const, \
             tc.tile_pool(name="persist", bufs=1) as persist:

            ident = const.tile([P, P], f32, tag="ident")
            make_identity(nc, ident)
            ident16 = const.tile([P, P], f16, tag="ident16")
            nc.scalar.copy(ident16, ident)
            masks_sb = const.tile([P, 4, 512], f16, tag="masks")
            for j in range(4):
                nc.sync.dma_start(masks_sb[:, j], masks_ap[j])
            bq_sb = const.tile([P, HC], f32, tag="bq")
            bk_sb = const.tile([P, HC], f32, tag="bk")
            bvb_sb = const.tile([P, HC * Dh], f32, tag="bvb")
            nc.sync.dma_start(bq_sb[:], bq.ap())
            nc.sync.dma_start(bk_sb[:], bk.ap())
            nc.sync.dma_start(bvb_sb[:], bvb.ap())

            kT_sb = persist.tile([P, HC, S], f32r, tag="kT")
            vaug_sb = persist.tile([P, HC, N_SC, VW], f16, tag="vaug")
            attnT_sb = persist.tile([P, HC, S], f16, tag="attnT")
            # ones column for softmax denominators
            nc.vector.memset(vaug_sb[:, :, :, Dh:Dh + 1], 1.0)

            with tc.tile_pool(name="abpool", bufs=1) as abpool, \
                 tc.tile_pool(name="wpool", bufs=CFG["WBUFS"]) as wpool, \
                 tc.tile_pool(name="exppool", bufs=CFG["EXPBUFS"]) as exppool, \
                 tc.tile_pool(name="spool", bufs=CFG["SPB"]) as spool, \
                 tc.tile_pool(name="qkps", bufs=CFG["QKPSB"], space="PSUM") as qkps, \
                 tc.tile_pool(name="trps", bufs=CFG["TRB"], space="PSUM") as trps, \
                 tc.tile_pool(name="qk2ps", bufs=CFG["QK2B"], space="PSUM") as qk2ps, \
                 tc.tile_pool(name="pvps", bufs=CFG["PVB"], space="PSUM") as pvps:

                for sf in range(SPLIT):
                    # ---- Phase A: projections for this S-chunk ----
                    xT_sb = abpool.tile([P, N_DC, SQ], f32r, tag="xT")
                    for dc in range(N_DC):
                        nc.sync.dma_start(
                            xT_sb[:, dc],
                            xT_ap[dc * P:(dc + 1) * P, sf * SQ:(sf + 1) * SQ])
                    qT_sb = abpool.tile([P, HC, SQ], f32r, tag="qT")
                    vT_sb = abpool.tile([P, HC, SQ], f16, tag="vT")

                    for h in range(HC):
                        for part, w_ap in (("q", wq_ap), ("k", wk_ap),
                                           ("v", wv_ap)):
                            wt = wpool.tile([P, N_DC, P], f32r, tag="w")
                            for wh in range(2):
                                nc.gpsimd.dma_start(
                                    wt[:, wh * 8:(wh + 1) * 8],
                                    w_ap[wh * 8 * P:(wh + 1) * 8 * P,
                                         h * P:(h + 1) * P].rearrange(
                                        "(dc p) c -> p dc c", p=P))
                            ps = qkps.tile([P, SQ], f32, tag="projps")
                            for dc in range(N_DC):
                                for st in range(SQ // 512):
                                    nc.tensor.matmul(
                                        ps[:, st * 512:(st + 1) * 512],
                                        wt[:, dc],
                                        xT_sb[:, dc, st * 512:(st + 1) * 512],
                                        start=(dc == 0), stop=(dc == N_DC - 1))
                            if part == "q":
                                nc.scalar.activation(
                                    qT_sb[:, h], ps,
                                    mybir.ActivationFunctionType.Identity,
                                    bias=bq_sb[:, h:h + 1])
                            elif part == "k":
                                nc.scalar.activation(
                                    kT_sb[:, h, sf * SQ:(sf + 1) * SQ], ps,
                                    mybir.ActivationFunctionType.Identity,
                                    bias=bk_sb[:, h:h + 1])
                            else:
                                nc.scalar.copy(vT_sb[:, h], ps)

                    # v: transpose to natural layout, add bias, cast to fp16
                    for h in range(HC):
                        for scl in range(SQ // P):
                            sc = sf * (SQ // P) + scl
                            tp = trps.tile([P, P], f16, tag="trv")
                            nc.tensor.transpose(
                                tp, vT_sb[:, h, scl * P:(scl + 1) * P], ident16)
                            nc.vector.tensor_add(
                                vaug_sb[:, h, sc, 0:Dh], tp,
                                bvb_sb[:, h * Dh:(h + 1) * Dh])

                    # ---- Phase B: attention for q rows in this S-chunk ----
                    for h in range(HC):
                      for qtl in range(SQ // 512):
                        qt = sf * (SQ // 512) + qtl
                        q0 = qt * 512
                        nk = (q0 + 512) // P
                        exps = []
                        for kc in range(nk):
                            j = kc - (nk - 4)
                            # columns < j*128 of this chunk are fully masked;
                            # narrow the matmul (f32r needs N >= 256, so the
                            # j == 3 chunk computes 256 wide instead of 128)
                            off = 0 if j < 1 else (256 if j == 3 else j * 128)
                            w = 512 - off
                            qk = qk2ps.tile([P, 512], f32, tag="qk")
                            nc.tensor.matmul(
                                qk[:, 0:w],
                                kT_sb[:, h, kc * P:(kc + 1) * P],
                                qT_sb[:, h,
                                      qtl * 512 + off:(qtl + 1) * 512],
                                start=True, stop=True)
                            ex = exppool.tile([P, 512], f16, tag="exp")
                            nc.scalar.activation(
                                ex[:, off:512], qk[:, 0:w],
                                mybir.ActivationFunctionType.Exp,
                                scale=SCALE)
                            if j >= 0:
                                nc.vector.tensor_mul(
                                    ex[:, off:512], ex[:, off:512],
                                    masks_sb[:, j, off:512])
                            exps.append(ex)
                        for sub in range(4):
                            nkq = qt * 4 + sub + 1
                            ps = pvps.tile([P, VW], f32, tag="pv")
                            for kc in range(nkq):
                                nc.tensor.matmul(
                                    ps[:, 0:Dh + 1],
                                    exps[kc][:, sub * P:(sub + 1) * P],
                                    vaug_sb[:, h, kc, 0:Dh + 1],
                                    start=(kc == 0),
                                    stop=(kc == nkq - 1))
                            rc = spool.tile([P, 1], f32, tag="rc")
                            nc.vector.reciprocal(rc, ps[:, Dh:Dh + 1])
                            at = spool.tile([P, P], f16, tag="at")
                            nc.vector.tensor_mul(
                                at, ps[:, 0:Dh], rc.to_broadcast((P, P)))
                            tp = trps.tile([P, P], f16, tag="tr")
                            nc.tensor.transpose(tp, at, ident16)
                            nc.scalar.copy(
                                attnT_sb[:, h,
                                         q0 + sub * P:q0 + (sub + 1) * P],
                                tp)

            # ---- Phase C: out projection (partial over this core's heads) ----
            with tc.tile_pool(name="wo_pool", bufs=1) as wo_pool, \
                 tc.tile_pool(name="osb", bufs=CFG["OSB"]) as osb, \
                 tc.tile_pool(name="ops", bufs=1, space="PSUM") as ops:
                wo_sb = wo_pool.tile([P, HC, D], f16, tag="wo")
                nc.gpsimd.dma_start(
                    wo_sb[:], wout_ap.rearrange("(hc p) d -> p hc d", p=P))
                for ss in range(N_SC):
                    ps = ops.tile([P, D], f32, tag="ops")
                    for hc in range(HC):
                        for n in range(D // 512):
                            nc.tensor.matmul(
                                ps[:, n * 512:(n + 1) * 512],
                                attnT_sb[:, hc, ss * P:(ss + 1) * P],
                                wo_sb[:, hc, n * 512:(n + 1) * 512],
                                start=(hc == 0),
                                stop=(hc == HC - 1))
                    ot = osb.tile([P, D], f32, tag="ot")
                    nc.scalar.copy(ot, ps)
                    nc.gpsimd.dma_start(outp_ap[ss * P:(ss + 1) * P, :], ot)

    nc.compile()
    return nc


def shard_inputs(x, Wqkv, bqkv, Wout):
    """Build the 8 per-core input maps."""
    mask = np.zeros((4, P, 512), np.float16)
    kk = np.arange(P)[:, None]
    qq = np.arange(512)[None, :]
    for j in range(4):
        mask[j] = (qq >= kk + P * j).astype(np.float16)

    in_maps = []
    for c in range(8):
        b, hg = divmod(c, 4)
        h0 = hg * HC                     # first head of this core
        c0 = h0 * Dh                     # column offset within a qkv part
        cw = HC * Dh                     # 512
        xT_c = np.ascontiguousarray(x[b].T)
        wq_c = np.ascontiguousarray(Wqkv[:, c0:c0 + cw])
        wk_c = np.ascontiguousarray(Wqkv[:, H * Dh + c0:H * Dh + c0 + cw])
        wv_c = np.ascontiguousarray(Wqkv[:, 2 * H * Dh + c0:2 * H * Dh + c0 + cw])
        wout_c = Wout[c0:c0 + cw, :].astype(np.float16)
        bq_c = np.ascontiguousarray(
            bqkv[c0:c0 + cw].reshape(HC, P).T).astype(np.float32)
        bk_c = np.ascontiguousarray(
            bqkv[H * Dh + c0:H * Dh + c0 + cw].reshape(HC, P).T).astype(np.float32)
        bv_c = bqkv[2 * H * Dh + c0:2 * H * Dh + c0 + cw].astype(np.float32)
        bvb_c = np.ascontiguousarray(np.broadcast_to(bv_c[None, :], (P, cw)))
        in_maps.append({
            "xT": xT_c, "wq": wq_c, "wk": wk_c, "wv": wv_c,
            "wout": wout_c, "bq": bq_c, "bk": bk_c, "bvb": bvb_c,
            "masks": mask,
        })
    return in_maps


def kernel(x, Wqkv, bqkv, Wout, bout):
    x = np.asarray(x, dtype=np.float32)
    Wqkv = np.asarray(Wqkv, dtype=np.float32)
    bqkv = np.asarray(bqkv, dtype=np.float32)
    Wout = np.asarray(Wout, dtype=np.float32)
    bout = np.asarray(bout, dtype=np.float32)

    if "nc" not in _COMPILED:
        _COMPILED["nc"] = build_program()
    nc = _COMPILED["nc"]

    in_maps = shard_inputs(x, Wqkv, bqkv, Wout)
    res = run_bass_kernel_spmd(nc, in_maps, core_ids=list(range(8)))

    out = np.empty((B, S, D), np.float32)
    for b in range(B):
        acc = res.results[4 * b]["outp"].astype(np.float32)
        for c in range(4 * b + 1, 4 * b + 4):
            acc = acc + res.results[c]["outp"]
        out[b] = acc + bout[None, :]
    return out
